# revision 24
# baseline (speedup 1.0000x reference)
"""NetVLAD consensus kernel for Trainium2 (8 NeuronCores, SPMD data-parallel).

Full-input contract: kernel(x, W, b, centroids) -> [32, 32768] fp32.

Sharding: data-parallel over batch N=32 -> 4 items per core; W/b/centroids
replicated. Items are processed in PAIRS stacked along the partition
dimension (item A on partitions 0..63, item B on 64..127) so every
K-dimension elementwise op uses all 128 ACT/DVE lanes; the matmul halves
target PE column-groups via PSUM base partitions. Per item:
  logitsT[k,t] = sum_c W[k,c] x[t,c]   (PE, contract C in 4 chunks of 128)
  e = exp(logitsT + b)                 (ACT, per-partition bias, per pair)
  eT tiles [t,k] via PE transpose; softmax normalize on DVE
  vlad[k,c] = sum_t a[t,k] x[t,c]      (PE, accumulate 8 t-tiles in PSUM)
  vlad -= asum*centroids; intra-L2-norm; global scale   (DVE per pair)

Key layout/perf choices:
- x is needed C-major for the logits contraction and T-major for the VLAD
  contraction, so the host passes both layouts in bf16, each in tile-major
  order (partition index outermost) so each load is 128 large contiguous
  descriptors.
- asum[k] = sum_t a[t,k] is folded into the VLAD matmul: the host appends a
  ones column to x and the VLAD matmul is split into N=257 / N=256 halves
  (PSUM bank limit). No separate N=1 matmuls.
- After intra-normalization every row has unit L2 norm, so the global norm
  equals sqrt(K) = 8 up to fp32 rounding (~1e-7); the final scale uses the
  constant 1/8.
- 1/norm = rsqrt(ss) is computed with the int bit-trick seed + two Newton
  steps on DVE (relative error ~5e-6, well under the bf16 noise floor),
  keeping the ACT engine on the Exp table the whole kernel.
"""

import numpy as np
import ml_dtypes
from contextlib import ExitStack

import concourse.bass as bass
import concourse.tile as tile
from concourse import bacc, mybir
from concourse.bass_utils import run_bass_kernel_spmd

N, T, C, K = 32, 1024, 512, 64
NCORES = 8
NB = N // NCORES          # batch items per core
NP = NB // 2              # item pairs per core
TT = 128                  # t-tile (partition dim for transposed ops)
TG = 512                  # t-group (logits matmul moving free dim)
NG = T // TG              # t-groups per item
NTT = T // TT             # t-tiles per item
NCC = C // 128            # c-chunks (contraction tiles)
CPAD = C + 2              # x augmented with a ones column (+ zero pad)
CA = C // 2 + 1           # first VLAD half: c 0..255 + asum column
CB = C // 2               # second VLAD half: c 256..511
EPS = 1e-12

f32 = mybir.dt.float32
bf16 = mybir.dt.bfloat16


def build_program(reps=1):
    """Build the SPMD Bass program (one core's view; same program all cores)."""
    nc = bacc.Bacc("TRN2", target_bir_lowering=False, debug=False,
                   num_devices=NCORES)

    x_d = nc.dram_tensor("x", [NB, 128, NTT, CPAD], bf16, kind="ExternalInput")
    xt_d = nc.dram_tensor("xT", [NB, NG, 128, NCC, TG], bf16, kind="ExternalInput")
    wt_d = nc.dram_tensor("WT", [C, K], bf16, kind="ExternalInput")
    b_d = nc.dram_tensor("b", [128, 1], f32, kind="ExternalInput")
    cent_d = nc.dram_tensor("cent", [128, C], f32, kind="ExternalInput")
    id_d = nc.dram_tensor("ident", [128, K], f32, kind="ExternalInput")
    out_d = nc.dram_tensor("out", [NB, K * C], f32, kind="ExternalOutput")

    with tile.TileContext(nc) as tc:
        with ExitStack() as ctx:
            _body(ctx, tc, nc, x_d, xt_d, wt_d, b_d, cent_d, id_d, out_d, reps)
    nc.compile()
    return nc


def _body(ctx, tc, nc, x_d, xt_d, wt_d, b_d, cent_d, id_d, out_d, reps):
    X = mybir.AxisListType.X
    Exp = mybir.ActivationFunctionType.Exp
    mult = mybir.AluOpType.mult
    add = mybir.AluOpType.add
    sub = mybir.AluOpType.subtract
    shr = mybir.AluOpType.arith_shift_right
    i32 = mybir.dt.int32
    HK = K  # 64: partition offset of the second item in a pair

    consts = ctx.enter_context(tc.tile_pool(name="consts", bufs=1))
    io = ctx.enter_context(tc.tile_pool(name="io", bufs=2))
    work = ctx.enter_context(tc.tile_pool(name="work", bufs=3))
    keep = ctx.enter_context(tc.tile_pool(name="keep", bufs=2))
    ps_vl = ctx.enter_context(tc.tile_pool(name="ps_vl", bufs=2, space="PSUM"))
    ps_lg = ctx.enter_context(tc.tile_pool(name="ps_lg", bufs=2, space="PSUM"))
    ps_eT = ctx.enter_context(tc.tile_pool(name="ps_eT", bufs=1, space="PSUM"))

    # --- constants.  The weight tile rides first on sync so the first logits
    # matmul can start as soon as possible; the small epilogue constants go
    # through gpsimd (SWDGE) to keep the HWDGE rings free for x. ---
    wt_sb = consts.tile([128, NCC, K], bf16)             # W^T c-chunks
    nc.sync.dma_start(wt_sb[:], wt_d.ap().rearrange("(cc p) k -> p cc k", p=128))
    b_sb = consts.tile([128, 1], f32)                    # [b; b]
    nc.gpsimd.dma_start(b_sb[:], b_d.ap())
    cent_sb = consts.tile([128, C], f32)                 # [cent; cent]
    nc.gpsimd.dma_start(cent_sb[:], cent_d.ap())
    id_sb = consts.tile([128, K], f32)                   # [I64; I64]
    nc.gpsimd.dma_start(id_sb[:], id_d.ap())
    magic = consts.tile([128, 1], i32)
    nc.vector.memset(magic[:], 0x5F3759DF)
    half3 = consts.tile([128, 1], f32)
    nc.vector.memset(half3[:], 1.5)

    for rep in range(reps):
        for p in range(NP):
            n0, n1 = 2 * p, 2 * p + 1
            xtb = [[io.tile([128, NCC, TG], bf16, tag=f"xtb{i}g{g}",
                    name=f"xtb{i}g{g}") for g in range(NG)] for i in (0, 1)]
            for g in range(NG):
                nc.scalar.dma_start(xtb[0][g][:], xt_d.ap()[n0, g])
                nc.scalar.dma_start(xtb[1][g][:], xt_d.ap()[n1, g])
            xb = [io.tile([128, NTT, CPAD], bf16, tag=f"xb{i}", name=f"xb{i}") for i in (0, 1)]
            nc.sync.dma_start(xb[0][:], x_d.ap()[n0])
            nc.sync.dma_start(xb[1][:], x_d.ap()[n1])

            # pair-stacked PSUM accumulators: item0 rows 0..63, item1 64..127
            vl_a = ps_vl.tile([128, CA], f32, tag="vl_a")
            vl_b = ps_vl.tile([128, CB], f32, tag="vl_b")
            eT = [ps_eT.tile([TT, NTT, K], f32, tag=f"eT{i}", name=f"eT{i}") for i in (0, 1)]
            a_sb = [work.tile([TT, NTT, K], bf16, tag=f"a{i}", name=f"a{i}") for i in (0, 1)]

            for g in range(NG):
                # logitsT [k, t-group]; both items accumulate into one bank
                # at different column-groups
                lg_ps = ps_lg.tile([128, TG], f32, tag="lg")
                for cc in range(NCC):
                    nc.tensor.matmul(
                        lg_ps[0:HK, :], wt_sb[:, cc, :],
                        xtb[0][g][:, cc, :],
                        start=(cc == 0), stop=(cc == NCC - 1))
                for cc in range(NCC):
                    nc.tensor.matmul(
                        lg_ps[HK:128, :], wt_sb[:, cc, :],
                        xtb[1][g][:, cc, :],
                        start=(cc == 0), stop=(cc == NCC - 1))
                # e = exp(logitsT + b) for the whole pair
                e_sb = work.tile([128, TG], f32, tag="e")
                nc.scalar.activation(e_sb[:], lg_ps[:], Exp, bias=b_sb[:])

                # transpose to [t, k] tiles (4 per group per item)
                for i in (0, 1):
                    lo, hi = i * HK, (i + 1) * HK
                    for j in range(TG // TT):
                        nc.tensor.transpose(
                            eT[i][:, g * (TG // TT) + j, :],
                            e_sb[lo:hi, bass.ts(j, TT)], id_sb[lo:hi, :])

            for i in (0, 1):
                # softmax normalize for one item: a = e / colsum(e)
                s_col = work.tile([TT, NTT, 1], f32, tag="s")
                nc.vector.reduce_sum(s_col[:], eT[i][:], axis=X)
                rs_col = work.tile([TT, NTT, 1], f32, tag="rs")
                nc.vector.reciprocal(rs_col[:], s_col[:])
                nc.vector.tensor_tensor(
                    out=a_sb[i][:], in0=eT[i][:],
                    in1=rs_col[:].broadcast_to([TT, NTT, K]), op=mult)

                # VLAD accumulation over t-tiles (split N=257/256; the ones
                # column of x makes vl_a[:, 256] the asum accumulator)
                lo, hi = i * HK, (i + 1) * HK
                for ti in range(NTT):
                    nc.tensor.matmul(
                        vl_a[lo:hi, :], a_sb[i][:, ti, :], xb[i][:, ti, 0:CA],
                        start=(ti == 0), stop=(ti == NTT - 1))
                    nc.tensor.matmul(
                        vl_b[lo:hi, :], a_sb[i][:, ti, :],
                        xb[i][:, ti, CA:CA + CB],
                        start=(ti == 0), stop=(ti == NTT - 1))

            # --- pair epilogue: centroid subtract + sum of squares ---
            nas = work.tile([128, 1], f32, tag="nas")
            nc.vector.tensor_scalar_mul(nas[:], vl_a[:, C // 2:C // 2 + 1], -1.0)
            vlad_sb = keep.tile([128, C], f32, tag="vlad")
            nc.vector.scalar_tensor_tensor(
                out=vlad_sb[:, 0:C // 2], in0=cent_sb[:, 0:C // 2],
                scalar=nas[:], in1=vl_a[:, 0:C // 2], op0=mult, op1=add)
            nc.vector.scalar_tensor_tensor(
                out=vlad_sb[:, C // 2:C], in0=cent_sb[:, C // 2:C],
                scalar=nas[:], in1=vl_b[:], op0=mult, op1=add)
            sq = work.tile([128, C], f32, tag="sq")
            ss = work.tile([128, 1], f32, tag="ss")
            nc.vector.scalar_tensor_tensor(
                out=sq[:], in0=vlad_sb[:], scalar=1.0, in1=vlad_sb[:],
                op0=mult, op1=mult, accum_out=ss[:])

            # rnorm = rsqrt(ss): bit-trick seed + two Newton steps (rel err
            # ~5e-6, far below the bf16 noise floor).  No ACT Sqrt -> the
            # activation table stays on Exp for the whole kernel.  After
            # intra-normalization each row is unit, so the global norm is
            # sqrt(K)=8 up to fp32 rounding; the final scale folds in 1/8.
            h = work.tile([128, 1], i32, tag="h")
            nc.vector.tensor_scalar(out=h[:], in0=ss[:].bitcast(i32),
                                    scalar1=1, scalar2=None, op0=shr)
            zb = work.tile([128, 1], i32, tag="zb")
            nc.vector.tensor_tensor(out=zb[:], in0=magic[:], in1=h[:], op=sub)
            ssn = work.tile([128, 1], f32, tag="ssn")
            nc.vector.tensor_scalar_mul(ssn[:], ss[:], -0.5)
            z = zb.bitcast(f32)
            for it in range(2):
                t2 = work.tile([128, 1], f32, tag="t2")
                nc.vector.tensor_tensor(out=t2[:], in0=z[:], in1=z[:], op=mult)
                v = work.tile([128, 1], f32, tag="v")
                nc.vector.scalar_tensor_tensor(
                    out=v[:], in0=t2[:], scalar=ssn[:], in1=half3[:],
                    op0=mult, op1=add)
                z2 = work.tile([128, 1], f32, tag="z2")
                nc.vector.tensor_tensor(out=z2[:], in0=z[:], in1=v[:], op=mult)
                z = z2

            outt = keep.tile([128, C], f32, tag="outt")
            nc.vector.tensor_scalar(
                out=outt[:], in0=vlad_sb[:], scalar1=z[:],
                scalar2=1.0 / 8.0, op0=mult, op1=mult)
            nc.gpsimd.dma_start(
                out_d.ap()[n0:n0 + 2].rearrange("n (k c) -> (n k) c", k=K),
                outt[:])


_NC_CACHE = {}


def _get_program(reps=1):
    if reps not in _NC_CACHE:
        _NC_CACHE[reps] = build_program(reps)
    return _NC_CACHE[reps]


def make_in_maps(x, W, b, centroids):
    x = np.asarray(x, dtype=np.float32)
    xaug = np.zeros((N, T, CPAD), dtype=ml_dtypes.bfloat16)
    xaug[:, :, :C] = x.astype(ml_dtypes.bfloat16)
    xaug[:, :, C] = 1.0
    # reorder so device slice [0:257] is c 0..255 + ones, [257:513] is c 256..511
    perm = list(range(C // 2)) + [C] + list(range(C // 2, C)) + [C + 1]
    xaug = xaug[:, :, perm]
    # tile-major: [N, 128, NTT, CPAD] with t = ti*128 + p
    xaug = np.ascontiguousarray(
        xaug.reshape(N, NTT, 128, CPAD).transpose(0, 2, 1, 3))
    xT = np.asarray(x.transpose(0, 2, 1)).astype(ml_dtypes.bfloat16)
    # group- and tile-major: [N, NG, 128, NCC, TG] with c = cc*128 + p,
    # t = g*TG + t'
    xT = np.ascontiguousarray(
        xT.reshape(N, NCC, 128, NG, TG).transpose(0, 3, 2, 1, 4))
    WT = np.ascontiguousarray(np.asarray(W, np.float32).T).astype(ml_dtypes.bfloat16)
    bcol = np.asarray(b, np.float32).reshape(K, 1)
    b2 = np.vstack([bcol, bcol])
    cent = np.asarray(centroids, np.float32)
    cent2 = np.vstack([cent, cent])
    ident = np.eye(K, dtype=np.float32)
    id2 = np.vstack([ident, ident])
    return [
        dict(x=xaug[i * NB:(i + 1) * NB], xT=xT[i * NB:(i + 1) * NB],
             WT=WT, b=b2, cent=cent2, ident=id2)
        for i in range(NCORES)
    ]


def kernel(x, W, b, centroids):
    nc = _get_program()
    in_maps = make_in_maps(x, W, b, centroids)
    res = run_bass_kernel_spmd(nc, in_maps, list(range(NCORES)))
    return np.concatenate([res.results[i]["out"] for i in range(NCORES)],
                          axis=0).reshape(N, K * C)
